# revision 34
# baseline (speedup 1.0000x reference)
"""Trainium2 Bass kernel for a dense transformer block.

Reference computation (per batch sample b):
    h   = LN(x; ln1_g, ln1_b)
    qkv = h @ qkv_w + qkv_b ; 16 heads, head_dim 64
    attn = softmax(q k^T / 8) v ; attn = attn @ out_w + out_b
    x2  = x + attn
    mlp = gelu_tanh(LN(x2; ln2) @ w1 + b1) @ w2 + b2
    out = x2 + mlp

Sharding: 8 cores; core c handles batch b=c//2 and query-token half
hf=c%2. The host rolls the sequence so each core's 1024 query tokens are
rows 0:1024 of its x input (attention keys are permutation invariant),
making the program SPMD-identical across cores. k/v are computed for the
full 2048-token sequence on both cores of a batch (small redundancy, no
collectives).

On-chip dataflow is "feature-major" (contraction dim on partitions)
throughout, so no large activation transposes are needed beyond the two
LayerNorm outputs (PE transposes of 128x128 tiles).
"""

import math
import os

import numpy as np
import ml_dtypes

H = 1024
NH = 16
HD = 64
I = 4096
B, S = 4, 2048
T = 2048          # tokens per core for k/v
Q = 1024          # query tokens per core
LN_EPS = 1e-5
P = 128

_CACHE = {}
LAST_RESULT = {}


def _build_program():
    import concourse.bass as bass
    import concourse.mybir as mybir
    import concourse.tile as tile
    from concourse import bacc
    from concourse.masks import make_identity
    from contextlib import ExitStack

    f32 = mybir.dt.float32
    bf16 = mybir.dt.bfloat16
    AF = mybir.ActivationFunctionType
    OP = mybir.AluOpType

    nc = bacc.Bacc(None, target_bir_lowering=False, debug=False)

    x_in = nc.dram_tensor("x_in", [T, H], f32, kind="ExternalInput")
    wqk_d = nc.dram_tensor("wqk", [H, 2 * H], bf16, kind="ExternalInput")
    bqk_d = nc.dram_tensor("bqk", [P, 16], f32, kind="ExternalInput")
    wv_d = nc.dram_tensor("wv", [H, H], bf16, kind="ExternalInput")
    bv_d = nc.dram_tensor("bv", [1, H], bf16, kind="ExternalInput")
    wout_d = nc.dram_tensor("wout", [H, H], bf16, kind="ExternalInput")
    bout_d = nc.dram_tensor("bout", [1, H], bf16, kind="ExternalInput")
    w1_d = nc.dram_tensor("w1", [H, I], bf16, kind="ExternalInput")
    b1_d = nc.dram_tensor("b1", [P, 32], f32, kind="ExternalInput")
    w2_d = nc.dram_tensor("w2", [I, H], bf16, kind="ExternalInput")
    b2_d = nc.dram_tensor("b2", [1, H], bf16, kind="ExternalInput")
    y_out = nc.dram_tensor("y", [Q, H], f32, kind="ExternalOutput")
    x2_d = nc.dram_tensor("x2_scratch", [Q, H], f32)
    h4_d = nc.dram_tensor("h4_scratch", [I // P, P, Q], bf16)

    # The kernel is split into sequential TileContexts. Each context exit is
    # a full drain + all-engine barrier with fresh semaphores after it, so
    # cross-phase dependencies ride on program order instead of semaphores.
    # This keeps every DMA's semaphore-wait count <= 2 (a hard walrus limit
    # on DMA descriptors): within a context no pool closes, so a DMA only
    # ever waits on its own tag's slot cycle (reader-release + same-lane
    # prior writer, aligned by choosing bufs so the reuse distance is a
    # multiple of the 8 DMA queues).
    es = ExitStack()
    with es:
        # kernel-lifetime SBUF tensors (cross-context)
        ident = es.enter_context(nc.sbuf_tensor([P, P], bf16))
        ones_row = es.enter_context(nc.sbuf_tensor([1, P], bf16))
        ones_bc = es.enter_context(nc.sbuf_tensor([P, HD], f32))
        eps_t = es.enter_context(nc.sbuf_tensor([P, 1], f32))
        bqk_s = es.enter_context(nc.sbuf_tensor([P, 16], f32))
        bv_row = es.enter_context(nc.sbuf_tensor([1, H], bf16))
        bout_row = es.enter_context(nc.sbuf_tensor([1, H], bf16))
        b1_s = es.enter_context(nc.sbuf_tensor([P, 32], f32))
        b2_row = es.enter_context(nc.sbuf_tensor([1, H], bf16))
        qT = es.enter_context(nc.sbuf_tensor([P, 8, Q], bf16))
        kT = es.enter_context(nc.sbuf_tensor([P, 8, T], bf16))
        v_aug = es.enter_context(nc.sbuf_tensor([P, 16, NH, HD + 1], bf16))
        attnT = es.enter_context(nc.sbuf_tensor([P, 8, Q], bf16))

        # ========== Single TileContext; phase-scoped pools ==========
        with tile.TileContext(nc) as tc, ExitStack() as c1:
            make_identity(nc, ident[:, :])
            nc.vector.memset(ones_row[:, :], 1.0)
            nc.vector.memset(ones_bc[:, :], 1.0)
            nc.vector.memset(eps_t[:, :], LN_EPS)
            nc.sync.dma_start(bqk_s[:, :], bqk_d[:, :])
            nc.sync.dma_start(bv_row[:, :], bv_d[:, :])
            nc.sync.dma_start(bout_row[:, :], bout_d[:, :])
            nc.sync.dma_start(b1_s[:, :], b1_d[:, :])
            nc.sync.dma_start(b2_row[:, :], b2_d[:, :])

            p_hT = c1.enter_context(tc.tile_pool(name="hT", bufs=1))
            hT = p_hT.tile([P, 8, T], bf16)
            esA = ExitStack()
            pA = esA.enter_context(tc.tile_pool(name="ln1", bufs=3))
            ps1 = esA.enter_context(
                tc.tile_pool(name="c1ps", bufs=4, space="PSUM"))

            # --- Phase A: LN1 + transpose to feature-major hT ---
            for t in range(T // P):
                # two half-loads per tile: 2 DMAs/iter x bufs=4 = reuse
                # distance 8, so the slot's prior writer shares this DMA's
                # queue and the WAW wait folds into queue order
                xtL = pA.tile([P, 512], f32, tag="xtL", bufs=4)
                nc.sync.dma_start(xtL, x_in[t * P:(t + 1) * P, 0:512])
                xtR = pA.tile([P, 512], f32, tag="xtR", bufs=4)
                nc.sync.dma_start(xtR, x_in[t * P:(t + 1) * P, 512:1024])
                st = pA.tile([P, 2, 6], f32, tag="st")
                nc.vector.bn_stats(st[:, 0, :], xtL)
                nc.vector.bn_stats(st[:, 1, :], xtR)
                mv = pA.tile([P, 2], f32, tag="mv")
                nc.vector.bn_aggr(mv, st)
                rstd = pA.tile([P, 1], f32, tag="rstd")
                nc.scalar.activation(rstd, mv[:, 1:2], AF.Sqrt, bias=eps_t[:, :])
                nc.vector.reciprocal(rstd, rstd)
                nmr = pA.tile([P, 1], f32, tag="nmr")
                nc.vector.tensor_tensor(
                    out=nmr, in0=mv[:, 0:1], in1=rstd, op=OP.mult)
                nc.vector.tensor_scalar_mul(nmr, nmr, -1.0)
                # normalize on ScalarE: x*rstd - mu*rstd (DVE is the
                # bottleneck engine in this phase)
                xh = pA.tile([P, H], bf16, tag="xh")
                nc.scalar.activation(
                    xh[:, 0:512], xtL, AF.Identity, bias=nmr, scale=rstd)
                nc.scalar.activation(
                    xh[:, 512:1024], xtR, AF.Identity, bias=nmr, scale=rstd)
                for hc in range(8):
                    pst = ps1.tile([P, P], bf16, tag="tp")
                    nc.tensor.transpose(pst, xh[:, hc * P:(hc + 1) * P],
                                        ident[:, :])
                    nc.vector.tensor_copy(hT[:, hc, t * P:(t + 1) * P], pst)

            # --- Phase C: v (token-major, with ones column for softmax
            # denominators) ---
            esA.close()
            esC = ExitStack()
            pC = esC.enter_context(tc.tile_pool(name="wv", bufs=1))
            psC = esC.enter_context(
                tc.tile_pool(name="vps", bufs=3, space="PSUM"))
            wv_s = pC.tile([P, 8, H], bf16)
            nc.sync.dma_start(wv_s, wv_d.rearrange("(c p) n -> p c n", p=P))
            nc.vector.memset(v_aug[:, :, :, HD:HD + 1], 1.0)
            for m in range(T // P):
                for nn in range(2):
                    ps = psC.tile([P, 512], f32, tag="ps", bufs=3)
                    for kc in range(8):
                        nc.tensor.matmul(
                            ps, lhsT=hT[:, kc, m * P:(m + 1) * P],
                            rhs=wv_s[:, kc, nn * 512:(nn + 1) * 512],
                            start=(kc == 0), stop=False)
                    nc.tensor.matmul(
                        ps, lhsT=ones_row[0:1, :],
                        rhs=bv_row[0:1, nn * 512:(nn + 1) * 512],
                        start=False, stop=True)
                    nc.vector.tensor_copy(
                        v_aug[:, m, nn * 8:(nn + 1) * 8, 0:HD],
                        ps.rearrange("p (h d) -> p h d", d=HD))

            esC.close()

            # ========== Phase B+D: qk projections interleaved with
            # attention ==========
            # The attention inner loop is ACT(exp)-bound; the qk projection
            # matmuls (pure PE work) are emitted between attention
            # iterations, one head-pair ahead, so the PE never idles long
            # enough for HAM to re-throttle it to 1.2 GHz.
            c2 = ExitStack()
            pB = c2.enter_context(tc.tile_pool(name="wqk", bufs=3))
            pD = c2.enter_context(tc.tile_pool(name="probs", bufs=2))
            pDs = c2.enter_context(tc.tile_pool(name="dsmall", bufs=2))
            psS = c2.enter_context(
                tc.tile_pool(name="sps", bufs=3, space="PSUM"))
            psV = c2.enter_context(
                tc.tile_pool(name="aps", bufs=2, space="PSUM"))

            bstate = {}

            def emit_b_group(nco, tcc):
                if bstate.get('nco') != nco:
                    wt = pB.tile([P, 8, P], bf16, tag="wt", bufs=3,
                                 name=f"wt_{nco}")
                    nc.sync.dma_start(
                        wt, wqk_d[:, nco * P:(nco + 1) * P]
                        .rearrange("(c p) n -> p c n", p=P))
                    bstate['nco'], bstate['wt'] = nco, wt
                wt = bstate['wt']
                ps = psS.tile([P, 1024], f32, tag="ps_s",
                              name=f"bps_{nco}_{tcc}")
                for kc in range(8):
                    nc.tensor.matmul(
                        ps[:, 0:512], lhsT=wt[:, kc, :],
                        rhs=hT[:, kc, tcc * 512:(tcc + 1) * 512],
                        start=(kc == 0), stop=(kc == 7))
                if nco < 8:
                    dst = qT[:, nco, tcc * 512:(tcc + 1) * 512]
                else:
                    dst = kT[:, nco - 8, tcc * 512:(tcc + 1) * 512]
                nc.scalar.activation(
                    dst, ps[:, 0:512], AF.Identity,
                    bias=bqk_s[:, nco:nco + 1])

            def b_units(hp):
                return ([(hp, t) for t in range(2)]
                        + [(8 + hp, t) for t in range(4)])
            def norm_tail(hc, off, qc, ps_a, drow):
                # broadcast 1/denom across partitions 64:128 of ps_a (the
                # denominator row there is dead once drow holds its
                # reciprocal) via a K=1 matmul
                nc.tensor.matmul(
                    ps_a[HD:P, :], lhsT=ones_bc[HD:HD + 1, :],
                    rhs=drow[HD:HD + 1, :], start=True, stop=True,
                    tile_position=(HD, HD))
                rb_s = pDs.tile([HD, 512], f32, tag="rb_s", name="rb_s")
                nc.vector.tensor_copy(rb_s, ps_a[HD:P, :])
                if off == 0:
                    nc.vector.tensor_mul(
                        attnT[0:HD, hc, qc * 512:(qc + 1) * 512],
                        ps_a[0:HD, :], rb_s)
                else:
                    # normalize, then shift partitions 0:64 -> 64:128 via an
                    # identity matmul (engines cannot cross partitions),
                    # borrowing a ps_s slot for the shifted copy
                    tmp = pDs.tile([HD, 512], bf16, tag="tmp", name="tmp")
                    nc.vector.tensor_mul(tmp, ps_a[0:HD, :], rb_s)
                    sh = psS.tile([P, 1024], f32, tag="ps_s", name="sh")
                    nc.tensor.matmul(
                        sh[HD:P, 0:512], lhsT=ident[0:HD, 0:HD], rhs=tmp,
                        start=True, stop=True, tile_position=(0, HD))
                    nc.vector.tensor_copy(
                        attnT[off:off + HD, hc, qc * 512:(qc + 1) * 512],
                        sh[HD:P, 0:512])

            # The normalization tail of iteration i is emitted after
            # iteration i+1's matmuls (software pipelining): its PE ops
            # (broadcast + shift) then never stall on the DVE reciprocal /
            # multiply chain, keeping the PE HAM-warm.
            for u in b_units(0):
                emit_b_group(*u)
            pend = None
            for hp in range(8):
                units = list(b_units(hp + 1)) if hp < 7 else []
                for qc in range(2):
                    hA, hB = 2 * hp, 2 * hp + 1
                    qThA = qT[0:HD, hp, qc * 512:(qc + 1) * 512]
                    qThB = qT[HD:P, hp, qc * 512:(qc + 1) * 512]
                    kThA = kT[0:HD, hp, :]
                    kThB = kT[HD:P, hp, :]
                    probsA = pD.tile([P, 16, 512], bf16, tag="probsA",
                                     name="probsA", bufs=2)
                    probsB = pD.tile([P, 16, 512], bf16, tag="probsB",
                                     name="probsB", bufs=1)
                    allot = 3
                    for g in range(8):
                        # interleave the head pair's score matmuls so
                        # consecutive matmuls hit different PE row groups
                        # (0:63 vs 64:127) and their LDWEIGHTS overlap in
                        # the 64-deep reorder window
                        psA = psS.tile([P, 1024], f32, tag="ps_s",
                                       name="psA")
                        psB = psS.tile([P, 1024], f32, tag="ps_s",
                                       name="psB")
                        for j in range(2):
                            kc = g * 2 + j
                            nc.tensor.matmul(
                                psA[:, j * 512:(j + 1) * 512],
                                lhsT=kThA[:, kc * P:(kc + 1) * P], rhs=qThA,
                                start=True, stop=True)
                            nc.tensor.matmul(
                                psB[:, j * 512:(j + 1) * 512],
                                lhsT=kThB[:, kc * P:(kc + 1) * P], rhs=qThB,
                                start=True, stop=True)
                        nc.scalar.activation(
                            probsA[:, g * 2:g * 2 + 2, :],
                            psA.rearrange("p (a b) -> p a b", b=512), AF.Exp)
                        nc.scalar.activation(
                            probsB[:, g * 2:g * 2 + 2, :],
                            psB.rearrange("p (a b) -> p a b", b=512), AF.Exp)
                        if g in (2, 4, 6) and allot > 0 and units:
                            emit_b_group(*units.pop(0))
                            allot -= 1
                    for h, probs in ((hA, probsA), (hB, probsB)):
                        off = (h % 2) * HD
                        ps_a = psV.tile([P, 512], f32, tag="ps_a",
                                        name="ps_a")
                        for kc in range(16):
                            nc.tensor.matmul(
                                ps_a[0:HD + 1, :], lhsT=v_aug[:, kc, h, :],
                                rhs=probs[:, kc, :],
                                start=(kc == 0), stop=(kc == 15))
                        drow = pDs.tile([P, 512], f32, tag="drow",
                                        name="drow")
                        nc.vector.reciprocal(drow[HD:HD + 1, :],
                                             ps_a[HD:HD + 1, :])
                        if units and allot > 0:
                            emit_b_group(*units.pop(0))
                            allot -= 1
                        if pend is not None:
                            norm_tail(*pend)
                        pend = (hp, off, qc, ps_a, drow)
                    for u in units[:allot]:
                        emit_b_group(*u)
                    units = units[allot:]
            norm_tail(*pend)
            c2.close()
            c1.close()  # hT pool lives until the end of the merged loop

            es_h2 = ExitStack()
            p_h2 = es_h2.enter_context(tc.tile_pool(name="h2Tp", bufs=1))
            h2T = p_h2.tile([P, 8, Q], bf16)

            # ========== Phase E: out_proj + residual + LN2 ==========
            c3 = ExitStack()
            pE = c3.enter_context(tc.tile_pool(name="oproj", bufs=3))
            pEw = c3.enter_context(tc.tile_pool(name="wout", bufs=1))
            psE = c3.enter_context(
                tc.tile_pool(name="ops", bufs=3, space="PSUM"))
            psEt = c3.enter_context(
                tc.tile_pool(name="tps", bufs=4, space="PSUM"))
            wout_s = pEw.tile([P, 8, H], bf16)
            nc.sync.dma_start(wout_s,
                                wout_d.rearrange("(c p) n -> p c n", p=P))
            for m in range(Q // P):
                xm = pE.tile([P, H], f32, tag="xm", bufs=4)
                nc.sync.dma_start(xm, x_in[m * P:(m + 1) * P, :])
                x2m = pE.tile([P, H], f32, tag="x2m")
                for nn in range(2):
                    ps = psE.tile([P, 512], f32, tag="ps")
                    for kc in range(8):
                        nc.tensor.matmul(
                            ps, lhsT=attnT[:, kc, m * P:(m + 1) * P],
                            rhs=wout_s[:, kc, nn * 512:(nn + 1) * 512],
                            start=(kc == 0), stop=False)
                    nc.tensor.matmul(
                        ps, lhsT=ones_row[0:1, :],
                        rhs=bout_row[0:1, nn * 512:(nn + 1) * 512],
                        start=False, stop=True)
                    nc.vector.tensor_add(
                        x2m[:, nn * 512:(nn + 1) * 512], ps,
                        xm[:, nn * 512:(nn + 1) * 512])
                nc.sync.dma_start(x2_d[m * P:(m + 1) * P, :], x2m)
                st = pE.tile([P, 2, 6], f32, tag="st")
                nc.vector.bn_stats(st[:, 0, :], x2m[:, 0:512])
                nc.vector.bn_stats(st[:, 1, :], x2m[:, 512:1024])
                mv = pE.tile([P, 2], f32, tag="mv")
                nc.vector.bn_aggr(mv, st)
                rstd = pE.tile([P, 1], f32, tag="rstd")
                nc.scalar.activation(rstd, mv[:, 1:2], AF.Sqrt,
                                     bias=eps_t[:, :])
                nc.vector.reciprocal(rstd, rstd)
                xh2 = pE.tile([P, H], bf16, tag="xh2")
                nc.vector.tensor_scalar(
                    out=xh2, in0=x2m, scalar1=mv[:, 0:1], scalar2=rstd,
                    op0=OP.subtract, op1=OP.mult)
                for hc2 in range(8):
                    pst = psEt.tile([P, P], bf16, tag="tp")
                    nc.tensor.transpose(pst, xh2[:, hc2 * P:(hc2 + 1) * P],
                                        ident[:, :])
                    nc.vector.tensor_copy(h2T[:, hc2, m * P:(m + 1) * P], pst)

            c3.close()

            # ========== Phase F1: MLP dense1 + gelu ==========
            c4 = ExitStack()
            pF1 = c4.enter_context(tc.tile_pool(name="w1p", bufs=3))
            pF1o = c4.enter_context(tc.tile_pool(name="h4o", bufs=3))
            psF1 = c4.enter_context(
                tc.tile_pool(name="m1ps", bufs=3, space="PSUM"))
            for mc in range(I // P):
                w1t = pF1.tile([P, 8, P], bf16, tag="w1t", bufs=4)
                nc.sync.dma_start(
                    w1t, w1_d[:, mc * P:(mc + 1) * P]
                    .rearrange("(c p) n -> p c n", p=P))
                h4t = pF1o.tile([P, Q], bf16, tag="h4t")
                for tcc in range(2):
                    ps = psF1.tile([P, 512], f32, tag="ps")
                    for kc in range(8):
                        nc.tensor.matmul(
                            ps, lhsT=w1t[:, kc, :],
                            rhs=h2T[:, kc, tcc * 512:(tcc + 1) * 512],
                            start=(kc == 0), stop=(kc == 7))
                    nc.scalar.activation(
                        h4t[:, tcc * 512:(tcc + 1) * 512], ps,
                        AF.Gelu_apprx_tanh, bias=b1_s[:, mc:mc + 1])
                nc.sync.dma_start(h4_d[mc, :, :], h4t)

            c4.close()
            es_h2.close()

            # ========== Phase F2: MLP dense2 + residual ==========
            c5 = ExitStack()
            pF2 = c5.enter_context(tc.tile_pool(name="w2p", bufs=3))
            pF2i = c5.enter_context(tc.tile_pool(name="h4i", bufs=3))
            pFy = c5.enter_context(tc.tile_pool(name="yp", bufs=4))
            psF2 = c5.enter_context(
                tc.tile_pool(name="m2ps", bufs=1, space="PSUM"))
            for nn in range(2):
                pss = [psF2.tile([P, 512], f32, tag=f"acc{m}", name=f"acc{m}")
                       for m in range(8)]
                for kc in range(I // P):
                    w2t = pF2.tile([P, 512], bf16, tag="w2t", bufs=4)
                    nc.sync.dma_start(
                        w2t, w2_d[kc * P:(kc + 1) * P,
                                  nn * 512:(nn + 1) * 512])
                    h4t = pF2i.tile([P, Q], bf16, tag="h4t", bufs=4)
                    nc.sync.dma_start(h4t, h4_d[kc, :, :])
                    for m in range(8):
                        nc.tensor.matmul(
                            pss[m], lhsT=h4t[:, m * P:(m + 1) * P],
                            rhs=w2t, start=(kc == 0), stop=False)
                for m in range(8):
                    nc.tensor.matmul(
                        pss[m], lhsT=ones_row[0:1, :],
                        rhs=b2_row[0:1, nn * 512:(nn + 1) * 512],
                        start=False, stop=True)
                    x2t = pFy.tile([P, 512], f32, tag="x2t", bufs=4)
                    nc.sync.dma_start(
                        x2t, x2_d[m * P:(m + 1) * P, nn * 512:(nn + 1) * 512])
                    yt = pFy.tile([P, 512], f32, tag="yt", bufs=4)
                    nc.vector.tensor_add(yt, pss[m], x2t)
                    nc.sync.dma_start(
                        y_out[m * P:(m + 1) * P, nn * 512:(nn + 1) * 512], yt)
            c5.close()

    nc.compile()
    return nc


def _prep_weights(inputs):
    f32 = np.float32
    bf = ml_dtypes.bfloat16
    ln1_g = np.asarray(inputs["ln1_g"], f32)
    ln1_b = np.asarray(inputs["ln1_b"], f32)
    qkv_w = np.asarray(inputs["qkv_w"], f32)
    qkv_b = np.asarray(inputs["qkv_b"], f32)
    ln2_g = np.asarray(inputs["ln2_g"], f32)
    ln2_b = np.asarray(inputs["ln2_b"], f32)
    w1 = np.asarray(inputs["w1"], f32)
    b1 = np.asarray(inputs["b1"], f32)

    wqkv = ln1_g[:, None] * qkv_w
    bqkv = ln1_b @ qkv_w + qkv_b
    wqkv[:, :H] *= 0.125          # fold 1/sqrt(head_dim) into q
    bqkv[:H] *= 0.125
    w1f = ln2_g[:, None] * w1
    b1f = ln2_b @ w1 + b1

    return {
        "wqk": np.ascontiguousarray(wqkv[:, :2 * H]).astype(bf),
        "bqk": np.ascontiguousarray(bqkv[:2 * H].reshape(16, P).T).astype(f32),
        "wv": np.ascontiguousarray(wqkv[:, 2 * H:]).astype(bf),
        "bv": bqkv[2 * H:].reshape(1, H).astype(bf),
        "wout": np.asarray(inputs["out_w"], f32).astype(bf),
        "bout": np.asarray(inputs["out_b"], f32).reshape(1, H).astype(bf),
        "w1": w1f.astype(bf),
        "b1": np.ascontiguousarray(b1f.reshape(32, P).T).astype(f32),
        "w2": np.asarray(inputs["w2"], f32).astype(bf),
        "b2": np.asarray(inputs["b2"], f32).reshape(1, H).astype(bf),
    }


def kernel(**inputs) -> np.ndarray:
    from concourse.bass_utils import run_bass_kernel_spmd

    if "nc" not in _CACHE:
        _CACHE["nc"] = _build_program()
    nc = _CACHE["nc"]

    x = np.asarray(inputs["x"], np.float32)
    weights = _prep_weights(inputs)

    in_maps = []
    for c in range(8):
        b, hf = c // 2, c % 2
        xb = x[b]
        if hf:
            xb = np.concatenate([xb[Q:], xb[:Q]], axis=0)
        m = {"x_in": np.ascontiguousarray(xb)}
        m.update(weights)
        in_maps.append(m)

    trace = os.environ.get("KERNEL_TRACE", "") == "1"
    res = run_bass_kernel_spmd(nc, in_maps, list(range(8)), trace=trace)
    LAST_RESULT["exec_time_ns"] = res.exec_time_ns
    LAST_RESULT["results"] = res

    out = np.empty((B, S, H), np.float32)
    for c in range(8):
        b, hf = c // 2, c % 2
        out[b, hf * Q:(hf + 1) * Q, :] = res.results[c]["y"]
    return out
